# revision 8
# baseline (speedup 1.0000x reference)
"""Distributed Trainium2 kernel for a dense-transformer attention block.

Math (matches the reference):
    xqkv = x @ Wqkv + bqkv ; split into q,k,v heads
    scores = (q k^T) / sqrt(HD) + mask ; attn = softmax(scores)
    o = attn @ v ; out = o @ Wproj + bproj

Parallelization over 8 NeuronCores (tensor-parallel over heads):
  - Each core owns NH/8 = 2 heads: computes the QKV projection for its
    heads only (weight-column shard), runs causal attention for those
    heads over the full batch/sequence, then an AllToAll redistributes
    the per-head outputs so each core holds ALL head-dims for 1/8 of the
    (batch*seq) rows.  Each core finishes with the output projection for
    its row block; the host concatenates row blocks.
  - The fp32->bf16 cast of x is data-parallel (1/8 each) + AllGather.

Layout notes:
  - Activations are kept transposed ([feature, token]) so the model dim
    lands on SBUF partitions for TensorE contraction; x^T tiles are
    produced with the DMA xbar transpose (needs 2-byte dtype -> bf16).
  - Scores are computed transposed (s^T[kv, q]) so softmax row-sums are
    matmuls with a ones-vector and attn@v consumes p^T directly.
  - Softmax skips max-subtraction (|scores| <= ~8 for this problem, and
    exp() is computed in fp32 out of PSUM).
  - The additive mask is analyzed on the host: fully-masked 512x512
    chunks are skipped, fully-visible chunks run unmasked, and mixed
    chunks get (deduplicated) mask tiles added to the score PSUM.
"""

import hashlib
import numpy as np
import ml_dtypes

B, S, DIM, NH = 4, 2048, 2048, 16
HD = DIM // NH  # 128
NCORES = 8
HPC = NH // NCORES          # heads per core = 2
TOK = B * S                 # 8192 tokens
RPC = TOK // NCORES         # rows (tokens) per core = 1024
CH = 512                    # attention chunk (q and kv)
SUB = 128                   # kv subtile
SCALE = 1.0 / float(np.sqrt(HD))

_BF16 = ml_dtypes.bfloat16

_prog_cache = {}


def _analyze_mask(mask):
    """Build the attention schedule from the additive mask.

    Returns (sched, mask_pack, tile_meta):
      sched[qc] = list of (kc, j, q_lo, mask_id, c_lo, c_hi); mask_id is
        -1 when no mask add is needed for the entry.
      mask_pack: [n_tiles, SUB, CH] bf16 array of mask tiles already
        transposed to [kv, q] layout and pre-divided by SCALE.
      tile widths are implied by (c_hi - c_lo) stored in the schedule.
    """
    m = np.asarray(mask, dtype=np.float32).reshape(S, S)
    NEG = -1e8
    nq, nk = S // CH, S // CH
    sched = []
    tiles = []          # list of np arrays [SUB, width]
    tile_key = {}       # content hash -> id
    for qc in range(nq):
        ents = []
        for kc in range(nk):
            blk = m[qc * CH:(qc + 1) * CH, kc * CH:(kc + 1) * CH]
            if np.all(blk <= NEG):
                continue
            for j in range(CH // SUB):
                sub = blk[:, j * SUB:(j + 1) * SUB]       # [CH q, SUB kv]
                if np.all(sub <= NEG):
                    continue
                vis = ~np.all(sub <= NEG, axis=1)         # per q row
                q_lo = int(np.argmax(vis))
                q_lo = (q_lo // SUB) * SUB
                if not ents:
                    q_lo = 0  # first entry must initialize full PSUM width
                nzrow = np.any(sub[q_lo:, :] != 0.0, axis=1)
                if nzrow.any():
                    first = q_lo + int(np.argmax(nzrow))
                    last = q_lo + len(nzrow) - int(np.argmax(nzrow[::-1]))
                    c_lo = (first // SUB) * SUB
                    c_hi = min(CH, ((last + SUB - 1) // SUB) * SUB)
                    content = np.ascontiguousarray(
                        (sub[c_lo:c_hi, :].T / SCALE).astype(_BF16))
                    key = (c_hi - c_lo,
                           hashlib.md5(content.tobytes()).hexdigest())
                    if key not in tile_key:
                        tile_key[key] = len(tiles)
                        tiles.append(content)
                    ents.append((kc, j, q_lo, tile_key[key], c_lo, c_hi))
                else:
                    ents.append((kc, j, q_lo, -1, 0, 0))
        assert ents, "a full query chunk is masked out; softmax undefined"
        sched.append(ents)
    n_real = len(tiles)
    widths = [t.shape[1] for t in tiles]
    pack = np.zeros((max(1, n_real), SUB, CH), dtype=_BF16)
    for i, t in enumerate(tiles):
        pack[i, :, :t.shape[1]] = t
    return sched, pack, widths, n_real


def _build_program(sched, n_mask_tiles, mask_widths):
    import concourse.bass as bass
    import concourse.tile as tile
    from concourse import bacc, mybir
    from contextlib import ExitStack

    f32 = mybir.dt.float32
    bf16 = mybir.dt.bfloat16
    AF = mybir.ActivationFunctionType
    ALU = mybir.AluOpType

    nc = bacc.Bacc("TRN2", target_bir_lowering=False, debug=False,
                   num_devices=NCORES)

    xs_ext = nc.dram_tensor("xs", [RPC, DIM], f32, kind="ExternalInput").ap()
    wqkv_ext = nc.dram_tensor("wqkv", [DIM, 3 * HPC * HD], bf16,
                              kind="ExternalInput").ap()
    bqkv_ext = nc.dram_tensor("bqkv", [3 * HPC * HD, 1], f32,
                              kind="ExternalInput").ap()
    maskt_ext = nc.dram_tensor("maskt", [max(1, n_mask_tiles), SUB, CH], bf16,
                               kind="ExternalInput").ap()
    wproj_ext = nc.dram_tensor("wproj", [DIM, DIM], bf16,
                               kind="ExternalInput").ap()
    bproj_ext = nc.dram_tensor("bproj", [1, DIM], f32,
                               kind="ExternalInput").ap()
    out_ext = nc.dram_tensor("out", [RPC, DIM], f32,
                             kind="ExternalOutput").ap()

    QKW = 3 * HPC * HD        # 768 projection output dims per core
    NDT = DIM // 128          # 16 contraction tiles
    rg = [list(range(NCORES))]

    with tile.TileContext(nc) as tc, ExitStack() as top:
        dram = top.enter_context(tc.tile_pool(name="dram", bufs=1,
                                              space="DRAM"))
        xg_local = dram.tile([RPC, DIM], bf16, name="xg_local")
        xg = dram.tile([TOK, DIM], bf16, name="xg", addr_space="Shared")
        a2a_in = dram.tile([DIM, RPC], bf16, name="a2a_in")
        a2a_out = dram.tile([DIM, RPC], bf16, name="a2a_out")

        const = top.enter_context(tc.tile_pool(name="const", bufs=1))
        ones = const.tile([128, 1], bf16, name="ones", tag="ones")
        nc.any.memset(ones[:], 1.0)
        # q/k per-partition biases (4 tiles of [128,1])
        bqk = []
        for t in range(2 * HPC):
            bt = const.tile([128, 1], f32, name=f"bqk{t}", tag=f"bqk{t}")
            nc.sync.dma_start(out=bt[:], in_=bqkv_ext[t * 128:(t + 1) * 128, :])
            bqk.append(bt)
        # v bias broadcast [128, HPC*HD]
        vb1 = const.tile([1, HPC * HD], f32, name="vb1", tag="vb1")
        nc.sync.dma_start(
            out=vb1[:],
            in_=bqkv_ext[2 * HPC * HD:3 * HPC * HD, :].rearrange("a b -> b a"))
        vbb = const.tile([128, HPC * HD], f32, name="vbb", tag="vbb")
        nc.gpsimd.partition_broadcast(vbb[:], vb1[:])
        # proj bias broadcast [128, DIM]
        bp1 = const.tile([1, DIM], f32, name="bp1", tag="bp1")
        nc.sync.dma_start(out=bp1[:], in_=bproj_ext[:, :])
        bpb = const.tile([128, DIM], f32, name="bpb", tag="bpb")
        nc.gpsimd.partition_broadcast(bpb[:], bp1[:])
        # mask tiles
        msk = []
        for i in range(n_mask_tiles):
            w = mask_widths[i]
            mt = const.tile([128, w], bf16, name=f"msk{i}", tag=f"msk{i}")
            nc.sync.dma_start(out=mt[:], in_=maskt_ext[i, :, :w])
            msk.append(mt)

        # ---- Phase 0: cast my x shard to bf16 and AllGather ----
        with tc.tile_pool(name="cast", bufs=2) as castp:
            for t in range(RPC // 128):
                xf = castp.tile([128, DIM], f32, name=f"xf{t}", tag="xf")
                nc.sync.dma_start(out=xf[:], in_=xs_ext[t * 128:(t + 1) * 128, :])
                xb = castp.tile([128, DIM], bf16, name=f"xb{t}", tag="xb")
                nc.vector.tensor_copy(xb[:], xf[:])
                nc.sync.dma_start(out=xg_local[t * 128:(t + 1) * 128, :],
                                  in_=xb[:])
        nc.gpsimd.collective_compute(
            "AllGather", mybir.AluOpType.bypass, replica_groups=rg,
            ins=[xg_local.opt()], outs=[xg.opt()])

        psA = top.enter_context(tc.tile_pool(name="psA", bufs=2, space="PSUM"))
        psV = top.enter_context(tc.tile_pool(name="psV", bufs=2, space="PSUM"))
        psO = top.enter_context(tc.tile_pool(name="psO", bufs=2, space="PSUM"))
        psS = top.enter_context(tc.tile_pool(name="psS", bufs=2, space="PSUM"))

        # persistent qkv storage (bf16)
        # qT/kT: [dh=128, S] per (b, local head); v: [kv_in_tile=128,
        # s_tile*HD] per (b, local head)
        qT = [[None] * HPC for _ in range(B)]
        kT = [[None] * HPC for _ in range(B)]
        vS = [[None] * HPC for _ in range(B)]
        frees = []
        for b in range(B):
            for h in range(HPC):
                t1, f1 = tc.tile([128, S], bf16, name=f"qT{b}{h}")
                t2, f2 = tc.tile([128, S], bf16, name=f"kT{b}{h}")
                t3, f3 = tc.tile([128, S], bf16, name=f"vS{b}{h}")
                qT[b][h], kT[b][h], vS[b][h] = t1, t2, t3
                frees += [f1, f2, f3]

        # ---- Phase 1: QKV projection ----
        with ExitStack() as p1:
            wpool = p1.enter_context(tc.tile_pool(name="wq", bufs=1))
            wq = []
            for dt_i in range(NDT):
                wt = wpool.tile([128, QKW], bf16, name=f"wq{dt_i}",
                                tag=f"wq{dt_i}")
                nc.sync.dma_start(
                    out=wt[:], in_=wqkv_ext[dt_i * 128:(dt_i + 1) * 128, :])
                wq.append(wt)
            xtp = p1.enter_context(tc.tile_pool(name="xtp", bufs=2))
            for b in range(B):
                for sc in range(S // CH):
                    row0 = b * S + sc * CH
                    xts = []
                    for dt_i in range(NDT):
                        xt = xtp.tile([128, CH], bf16, name=f"xt{dt_i}",
                                      tag=f"xt{dt_i}")
                        nc.sync.dma_start_transpose(
                            xt[:],
                            xg[row0:row0 + CH, dt_i * 128:(dt_i + 1) * 128])
                        xts.append(xt)
                    # q^T / k^T: 4 output tiles of [128, CH]
                    for t in range(2 * HPC):
                        ps = psA.tile([128, CH], f32, name="psqk", tag="A")
                        for dt_i in range(NDT):
                            nc.tensor.matmul(
                                ps[:], wq[dt_i][:, t * 128:(t + 1) * 128],
                                xts[dt_i][:],
                                start=(dt_i == 0), stop=(dt_i == NDT - 1))
                        dst = (qT if t < HPC else kT)[b][t % HPC]
                        nc.vector.tensor_scalar_add(
                            dst[:, sc * CH:(sc + 1) * CH], ps[:], bqk[t][:])
                    # v natural: 4 s-tiles of [128, HPC*HD]
                    for st in range(CH // 128):
                        ps = psV.tile([128, HPC * HD], f32, name="psv", tag="V")
                        for dt_i in range(NDT):
                            nc.tensor.matmul(
                                ps[:],
                                xts[dt_i][:, st * 128:(st + 1) * 128],
                                wq[dt_i][:, 2 * HPC * HD:3 * HPC * HD],
                                start=(dt_i == 0), stop=(dt_i == NDT - 1))
                        gst = sc * (CH // 128) + st  # global s tile idx
                        for h in range(HPC):
                            nc.vector.scalar_tensor_tensor(
                                out=vS[b][h][:, gst * HD:(gst + 1) * HD],
                                in0=ps[:, h * HD:(h + 1) * HD],
                                scalar=1.0,
                                in1=vbb[:, h * HD:(h + 1) * HD],
                                op0=ALU.mult, op1=ALU.add)

            # ---- Phase 2: attention per (b, local head) ----
            ptp = p1.enter_context(tc.tile_pool(name="ptp", bufs=3))
            recp = p1.enter_context(tc.tile_pool(name="recp", bufs=2))
            bcp = p1.enter_context(tc.tile_pool(name="bcp", bufs=2))
            otp = p1.enter_context(tc.tile_pool(name="otp", bufs=3))
            for b in range(B):
                for h in range(HPC):
                    for qc in range(S // CH):
                        ents = sched[qc]
                        o_ps = psO.tile([128, CH], f32, name="o_ps", tag="O")
                        s_sum = psS.tile([1, CH], f32, name="s_sum", tag="Ssum")
                        last = len(ents) - 1
                        for ei, (kc, j, q_lo, mid, c_lo, c_hi) in enumerate(ents):
                            kv0 = kc * CH + j * SUB
                            w = CH - q_lo
                            sps = psA.tile([128, CH], f32, name="sps", tag="A")
                            nc.tensor.matmul(
                                sps[:, q_lo:CH],
                                kT[b][h][:, kv0:kv0 + SUB],
                                qT[b][h][:, qc * CH + q_lo:(qc + 1) * CH],
                                start=True, stop=True)
                            if mid >= 0:
                                nc.vector.tensor_add(
                                    sps[:, c_lo:c_hi], sps[:, c_lo:c_hi],
                                    msk[mid][:, :c_hi - c_lo])
                            pT = ptp.tile([128, CH], bf16, name="pT", tag="pT")
                            nc.scalar.activation(
                                pT[:, q_lo:CH], sps[:, q_lo:CH], AF.Exp,
                                scale=SCALE)
                            nc.tensor.matmul(
                                s_sum[:, q_lo:CH], ones[:], pT[:, q_lo:CH],
                                start=(ei == 0), stop=(ei == last))
                            kvt = kc * (CH // SUB) + j
                            nc.tensor.matmul(
                                o_ps[:, q_lo:CH],
                                vS[b][h][:, kvt * HD:(kvt + 1) * HD],
                                pT[:, q_lo:CH],
                                start=(ei == 0), stop=(ei == last))
                        rec = recp.tile([1, CH], f32, name="rec", tag="rec")
                        nc.vector.reciprocal(rec[:], s_sum[:])
                        bc = bcp.tile([128, CH], f32, name="bc", tag="bc")
                        nc.gpsimd.partition_broadcast(bc[:], rec[:])
                        oT = otp.tile([128, CH], bf16, name="oT", tag="oT")
                        nc.vector.tensor_mul(oT[:], o_ps[:], bc[:])
                        dest = 2 * b + qc // 2
                        r0 = dest * (HPC * HD) + h * HD
                        col0 = (qc % 2) * CH
                        nc.sync.dma_start(
                            out=a2a_in[r0:r0 + HD, col0:col0 + CH], in_=oT[:])

        for f in reversed(frees):
            f()

        nc.gpsimd.collective_compute(
            "AllToAll", mybir.AluOpType.bypass, replica_groups=rg,
            ins=[a2a_in.opt()], outs=[a2a_out.opt()])

        # ---- Phase 4: output projection for my RPC rows ----
        with ExitStack() as p4:
            ocp = p4.enter_context(tc.tile_pool(name="ocp", bufs=1))
            oc = []
            for ot in range(NDT):
                t = ocp.tile([128, RPC], bf16, name=f"oc{ot}", tag=f"oc{ot}")
                nc.sync.dma_start(out=t[:],
                                  in_=a2a_out[ot * 128:(ot + 1) * 128, :])
                oc.append(t)
            wpp = p4.enter_context(tc.tile_pool(name="wpp", bufs=2))
            resp = p4.enter_context(tc.tile_pool(name="resp", bufs=3))
            for nchunk in range(DIM // CH):
                wps = []
                for ot in range(NDT):
                    wp = wpp.tile([128, CH], bf16, name=f"wp{ot}",
                                  tag=f"wp{ot}")
                    nc.sync.dma_start(
                        out=wp[:],
                        in_=wproj_ext[ot * 128:(ot + 1) * 128,
                                      nchunk * CH:(nchunk + 1) * CH])
                    wps.append(wp)
                for rt in range(RPC // 128):
                    ps = psA.tile([128, CH], f32, name="pso", tag="A")
                    for ot in range(NDT):
                        nc.tensor.matmul(
                            ps[:], oc[ot][:, rt * 128:(rt + 1) * 128],
                            wps[ot][:],
                            start=(ot == 0), stop=(ot == NDT - 1))
                    res = resp.tile([128, CH], f32, name="res", tag="res")
                    nc.vector.scalar_tensor_tensor(
                        out=res[:], in0=ps[:], scalar=1.0,
                        in1=bpb[:, nchunk * CH:(nchunk + 1) * CH],
                        op0=ALU.mult, op1=ALU.add)
                    nc.sync.dma_start(
                        out=out_ext[rt * 128:(rt + 1) * 128,
                                    nchunk * CH:(nchunk + 1) * CH],
                        in_=res[:])

    nc.compile()
    return nc


def _get_program(sched, n_real, mask_widths):
    key = (str(sched), tuple(mask_widths))
    if key not in _prog_cache:
        _prog_cache[key] = _build_program(sched, n_real, mask_widths)
    return _prog_cache[key]


def kernel(x=None, mask=None, Wqkv=None, bqkv=None, Wproj=None, bproj=None,
           start_pos=0, **_unused):
    from concourse.bass_utils import run_bass_kernel_spmd

    x = np.asarray(x, dtype=np.float32).reshape(TOK, DIM)
    mask = np.asarray(mask, dtype=np.float32)
    Wqkv = np.asarray(Wqkv, dtype=np.float32)
    bqkv = np.asarray(bqkv, dtype=np.float32)
    Wproj = np.asarray(Wproj, dtype=np.float32)
    bproj = np.asarray(bproj, dtype=np.float32)

    sched, mask_pack, widths, n_real = _analyze_mask(mask)
    nc = _get_program(sched, n_real, widths)

    wproj_bf = np.ascontiguousarray(Wproj.astype(_BF16))
    bproj2 = np.ascontiguousarray(bproj.reshape(1, DIM))

    in_maps = []
    for c in range(NCORES):
        heads = [HPC * c + i for i in range(HPC)]
        cols = []
        for grp in range(3):  # q, k, v column groups
            for hh in heads:
                c0 = grp * DIM + hh * HD
                cols.append((c0, c0 + HD))
        w_sh = np.concatenate([Wqkv[:, a:b] for a, b in cols], axis=1)
        b_sh = np.concatenate([bqkv[a:b] for a, b in cols])
        in_maps.append({
            "xs": np.ascontiguousarray(x[c * RPC:(c + 1) * RPC]),
            "wqkv": np.ascontiguousarray(w_sh.astype(_BF16)),
            "bqkv": np.ascontiguousarray(b_sh.reshape(-1, 1)),
            "maskt": mask_pack,
            "wproj": wproj_bf,
            "bproj": bproj2,
        })

    import os
    kw = {}
    if os.environ.get("KERNEL_TRACE"):
        kw["trace"] = True
    res = run_bass_kernel_spmd(nc, in_maps, core_ids=list(range(NCORES)), **kw)
    globals()["LAST_RUN"] = res
    if getattr(res, "exec_time_ns", None):
        print(f"HW exec time: {res.exec_time_ns} ns")
    outs = [res.results[c]["out"] for c in range(NCORES)]
    full = np.concatenate(outs, axis=0).reshape(B, S, DIM)
    return full.astype(np.float32)
